# revision 1
# baseline (speedup 1.0000x reference)
"""Grouped gated DeltaNet (KDA-style) on 8 TRN2 NeuronCores.

Sharding: core c -> (batch b = c//4, head-group hg = c%4 of 4 heads).
Each core: column-sharded projections, short-conv+silu, l2norm, chunked
gated delta-rule recurrence (chunk C=128, group-factorized per-channel
decay, triangular solve by Neumann doubling on TensorE), gated RMSNorm,
row-shard of the output projection. Host sums 4 partials per batch.

Self-contained: B=2, T=1024, D=2048, H=16, DK=DV=128 hardcoded.
"""
import sys
sys.path.insert(0, '/opt/trn_rl_repo')
import numpy as np
import ml_dtypes
from contextlib import ExitStack

B, T, D = 2, 1024, 2048
H, DK, DV, GG = 16, 128, 128, 16
NG = DK // GG          # 8 gate groups per head
NH = 4                 # heads per core
C = 128                # chunk length
NCH = T // C
SCALE = DK ** -0.5
EPS = 1e-5

BF = ml_dtypes.bfloat16
_CACHE = {}


def _build():
    import concourse.tile as tile
    from concourse import bacc, mybir

    fp32 = mybir.dt.float32
    bf16 = mybir.dt.bfloat16
    Alu = mybir.AluOpType
    Act = mybir.ActivationFunctionType

    nc = bacc.Bacc("TRN2", target_bir_lowering=False, debug=False, num_devices=8)
    dp = lambda n, sh, dt: nc.dram_tensor(n, sh, dt, kind="ExternalInput").ap()
    hT = dp("hT", [D, T], bf16)
    wq = dp("wq", [D, NH * DK], bf16)
    wk = dp("wk", [D, NH * DK], bf16)
    wv = dp("wv", [D, NH * DV], bf16)
    wg = dp("wg", [D, NH * DV], bf16)
    wo = dp("wo", [NH * DV, D], bf16)
    wf1 = dp("wf1", [D, DV], bf16)
    wf2 = dp("wf2", [DV, NH * NG], bf16)
    wb = dp("wb", [D, NH], bf16)
    cw = dp("cw", [NH * DK, 12], fp32)
    nega = dp("nega", [NH * NG, 1], fp32)
    dtb = dp("dtb", [NH * NG, 1], fp32)
    bgc = dp("bgc", [DV, NH], fp32)
    normw = dp("normw", [DV, 1], fp32)
    repl = dp("repl", [NG, DK], fp32)
    self8f = dp("self8f", [NG, NG * C], fp32)
    ones1b = dp("ones1b", [1, C], bf16)
    onescol = dp("onescol", [DK, 1], bf16)
    oh8 = dp("oh8", [DK, 64], bf16)
    sel8b = dp("sel8b", [8, 8 * 128], bf16)
    gmc = dp("gmc", [DK, NG], fp32)
    sc8 = dp("sc8", [8, 1], fp32)
    eps8 = dp("eps8", [8, 1], fp32)
    epsn = dp("epsn", [1, 1], fp32)
    maskM = dp("maskM", [C, C], bf16)
    maskG = dp("maskG", [C, C], bf16)
    idbf = dp("idbf", [128, 128], bf16)
    idf32 = dp("idf32", [128, 128], fp32)
    outT = nc.dram_tensor("outT", [D, T], fp32, kind="ExternalOutput").ap()

    with tile.TileContext(nc) as tc, ExitStack() as ctx:
        pool = lambda name, bufs, space="SBUF": ctx.enter_context(
            tc.tile_pool(name=name, bufs=bufs, space=space))

        cons = pool("cons", 1)
        htp = pool("htp", 1)
        wst = pool("wst", 1)
        wsm = pool("wsm", 1)
        pers = pool("pers", 1)
        convp = pool("convp", 1)
        sqp = pool("sqp", 1)
        chk = pool("chk", 2)
        big = pool("big", 1)
        st = pool("st", 1)
        # PSUM: 8 banks total.  proj(2) + big(2) + sm1(2) + gp(1) + otp(1)
        pps = pool("pps", 2, "PSUM")
        pbig = pool("pbig", 1, "PSUM")
        psm = pool("psm", 1, "PSUM")

        def sm1(shape, dt=fp32):
            return psm.tile(shape, dt, tag="sm1", bufs=2, name="sm1t")

        dma = nc.sync.dma_start

        # ---- consts ----
        cwt = []
        for m in range(4):
            t = cons.tile([128, 12], fp32, tag=f"cw{m}", name=f"cw{m}")
            dma(t[:], cw[m * 128:(m + 1) * 128, :])
            cwt.append(t)

        def ctile(shape, dt, src, nm):
            t = cons.tile(shape, dt, tag=nm, name=nm)
            dma(t[:], src[:])
            return t
        negat = ctile([32, 1], fp32, nega, "negat")
        dtbt = ctile([32, 1], fp32, dtb, "dtbt")
        bgt = ctile([128, 4], fp32, bgc, "bgt")
        nwt = ctile([128, 1], fp32, normw, "nwt")
        replt = ctile([8, 128], fp32, repl, "replt")
        s8f = ctile([NG, NG * C], fp32, self8f, "s8f")
        o1b = ctile([1, C], bf16, ones1b, "o1b")
        oct_ = ctile([128, 1], bf16, onescol, "oct")
        oh8t = ctile([128, 64], bf16, oh8, "oh8t")
        s8b = ctile([8, 8 * 128], bf16, sel8b, "s8b")
        gmct = ctile([128, NG], fp32, gmc, "gmct")
        sc8t = ctile([8, 1], fp32, sc8, "sc8t")
        eps8t = ctile([8, 1], fp32, eps8, "eps8t")
        epsnt = ctile([1, 1], fp32, epsn, "epsnt")
        mMt = ctile([128, 128], bf16, maskM, "mMt")
        mGt = ctile([128, 128], bf16, maskG, "mGt")
        idb = ctile([128, 128], bf16, idbf, "idb")
        idf = ctile([128, 128], fp32, idf32, "idf")
        ones32 = cons.tile([32, C], fp32, tag="ones32", name="ones32")
        nc.vector.memset(ones32[:], 1.0)

        # ---- hidden^T resident ----
        ht = []
        for k in range(16):
            t = htp.tile([128, T], bf16, tag=f"ht{k}", name=f"ht{k}")
            dma(t[:], hT[k * 128:(k + 1) * 128, :])
            ht.append(t)

        # ---- projections ----
        mk = lambda p, nm, dt=bf16, sh=None: [
            p.tile(sh or [128, T], dt, tag=f"{nm}{m}", name=f"{nm}{m}") for m in range(4)]
        qb, kb, vb = mk(pers, "qb"), mk(pers, "kb"), mk(pers, "vb")
        gateb, yb = mk(pers, "gateb"), mk(pers, "yb")
        f1b = pers.tile([128, T], bf16, tag="f1b", name="f1b")
        qs = {}
        ssqsb = cons.tile([8, T], fp32, tag="ssqsb", name="ssqsb")
        nc.vector.memset(ssqsb[:], 0.0)

        def project(w_ap, m, dst_bf16=None, conv_slot=None, pair=None, gate_bias=None):
            wt = [wst.tile([128, 512], bf16, tag=f"w{k}", name=f"wt{k}")
                  for k in range(16)]
            for k in range(16):
                dma(wt[k][:], w_ap[k * 128:(k + 1) * 128, :])
            xpad = None
            if conv_slot is not None:
                xpad = convp.tile([128, T + 3], fp32, tag="xpad", name="xpad", bufs=2)
                nc.vector.memset(xpad[:, 0:3], 0.0)
            for half in range(2):
                ps = pps.tile([128, 512], fp32, tag="proj", name="projps")
                for k in range(16):
                    nc.tensor.matmul(ps[:], wt[k][:, m * 128:(m + 1) * 128],
                                     ht[k][:, half * 512:(half + 1) * 512],
                                     start=(k == 0), stop=(k == 15))
                if xpad is not None:
                    nc.scalar.copy(xpad[:, 3 + half * 512: 3 + (half + 1) * 512], ps[:])
                elif gate_bias is not None:
                    nc.scalar.activation(dst_bf16[:, half * 512:(half + 1) * 512],
                                         ps[:], Act.Silu, bias=gate_bias)
                else:
                    nc.scalar.copy(dst_bf16[:, half * 512:(half + 1) * 512], ps[:])
            if xpad is None:
                return
            cwm = cwt[m]
            s = conv_slot * 4
            a = convp.tile([128, T], fp32, tag="acca", name="acca")
            bt = convp.tile([128, T], fp32, tag="accb", name="accb")
            nc.vector.tensor_scalar(a[:], xpad[:, 3:3 + T], cwm[:, s + 3:s + 4], None,
                                    op0=Alu.mult)
            cur, nxt = a, bt
            for kk in (2, 1, 0):
                nc.vector.scalar_tensor_tensor(nxt[:], xpad[:, kk:kk + T],
                                               cwm[:, s + kk:s + kk + 1], cur[:],
                                               op0=Alu.mult, op1=Alu.add)
                cur, nxt = nxt, cur
            if pair is None:
                nc.scalar.activation(dst_bf16[:], cur[:], Act.Silu)
            else:
                qsil = qb[pair] if pair < 4 else kb[pair - 4]
                qs[pair] = qsil
                nc.scalar.activation(qsil[:], cur[:], Act.Silu)
                sq = sqp.tile([128, T], bf16, tag="sq", name="sq")
                nc.scalar.activation(sq[:], qsil[:], Act.Square)
                for half in range(2):
                    pss = sm1([8, 512])
                    nc.tensor.matmul(pss[:], oh8t[:, pair * 8:pair * 8 + 8],
                                     sq[:, half * 512:(half + 1) * 512],
                                     start=True, stop=True)
                    nc.vector.tensor_tensor(ssqsb[:, half * 512:(half + 1) * 512],
                                            ssqsb[:, half * 512:(half + 1) * 512],
                                            pss[:], op=Alu.add)

        for m in range(4):
            project(wq, m, conv_slot=0, pair=m)
        for m in range(4):
            project(wk, m, conv_slot=1, pair=4 + m)
        for m in range(4):
            project(wv, m, dst_bf16=vb[m], conv_slot=2)
        for m in range(4):
            project(wg, m, dst_bf16=gateb[m], gate_bias=bgt[:, m:m + 1])

        # l2 normalizers
        nrm = cons.tile([8, T], fp32, tag="nrm", name="nrm")
        rec = cons.tile([8, T], fp32, tag="ssqsb", name="rec")
        recb = cons.tile([8, T], bf16, tag="recb", name="recb")
        nc.scalar.activation(nrm[:], ssqsb[:], Act.Ln, scale=sc8t[:, 0:1],
                             bias=eps8t[:, 0:1])
        nc.scalar.activation(recb[:], nrm[:], Act.Exp, scale=-0.5)
        for pair in range(8):
            dst = qb[pair] if pair < 4 else kb[pair - 4]
            for half in range(2):
                nb = sm1([128, 512])
                nc.tensor.matmul(nb[:], s8b[:, pair * 128:(pair + 1) * 128],
                                 recb[:, half * 512:(half + 1) * 512],
                                 start=True, stop=True)
                nc.vector.tensor_tensor(dst[:, half * 512:(half + 1) * 512],
                                        qs[pair][:, half * 512:(half + 1) * 512],
                                        nb[:], op=Alu.mult)

        # ---- f / beta ----
        wt1 = [wsm.tile([128, 128], bf16, tag=f"wf1_{k}", name=f"wf1_{k}")
               for k in range(16)]
        for k in range(16):
            dma(wt1[k][:], wf1[k * 128:(k + 1) * 128, :])
        for half in range(2):
            ps = pps.tile([128, 512], fp32, tag="proj", name="f1ps")
            for k in range(16):
                nc.tensor.matmul(ps[:], wt1[k][:], ht[k][:, half * 512:(half + 1) * 512],
                                 start=(k == 0), stop=(k == 15))
            nc.scalar.copy(f1b[:, half * 512:(half + 1) * 512], ps[:])
        wf2t = wsm.tile([128, 32], bf16, tag="wf2t", name="wf2t")
        dma(wf2t[:], wf2[:])
        wbt = [wsm.tile([128, 4], bf16, tag=f"wb{k}", name=f"wbt{k}")
               for k in range(16)]
        for k in range(16):
            dma(wbt[k][:], wb[k * 128:(k + 1) * 128, :])
        gna = cons.tile([32, T], fp32, tag="gna", name="gna")
        bsg = cons.tile([4, T], fp32, tag="bsg", name="bsg")
        for half in range(2):
            gps = sm1([32, 512])
            nc.tensor.matmul(gps[:], wf2t[:], f1b[:, half * 512:(half + 1) * 512],
                             start=True, stop=True)
            spe = chk.tile([32, 512], fp32, tag="spe", name="spe")
            nc.scalar.activation(spe[:], gps[:], Act.Exp, bias=dtbt[:, 0:1])
            sp1 = chk.tile([32, 512], fp32, tag="sp", name="sp1")
            nc.vector.tensor_scalar(sp1[:], spe[:], 1.0, None, op0=Alu.add)
            sp = chk.tile([32, 512], fp32, tag="spe", name="sp")
            nc.scalar.activation(sp[:], sp1[:], Act.Ln)
            nc.vector.tensor_scalar(gna[:, half * 512:(half + 1) * 512], sp[:],
                                    negat[:, 0:1], None, op0=Alu.mult)
            bps = sm1([4, 512])
            for k in range(16):
                nc.tensor.matmul(bps[:], wbt[k][:], ht[k][:, half * 512:(half + 1) * 512],
                                 start=(k == 0), stop=(k == 15))
            nc.scalar.activation(bsg[:, half * 512:(half + 1) * 512], bps[:], Act.Sigmoid)

        # ---- recurrence ----
        Sf = [st.tile([128, 128], fp32, tag=f"Sf{h}", name=f"Sf{h}") for h in range(4)]
        Sb = [st.tile([128, 128], bf16, tag=f"Sb{h}", name=f"Sb{h}") for h in range(4)]
        for h in range(4):
            nc.vector.memset(Sf[h][:], 0.0)
            nc.vector.memset(Sb[h][:], 0.0)

        for ci in range(NCH):
            ts = slice(ci * C, (ci + 1) * C)
            cN32 = chk.tile([32, C], fp32, tag="cN32", name="cN32")
            nc.vector.tensor_tensor_scan(cN32[:], ones32[:], gna[:, ts], 0.0,
                                         op0=Alu.mult, op1=Alu.add)
            cntp = sm1([128, 32])
            nc.tensor.transpose(cntp[:], cN32[:], idf[0:32, 0:32])
            cNt = chk.tile([128, 32], fp32, tag="cNt", name="cNt")
            nc.scalar.copy(cNt[:], cntp[:])
            cN8s = []
            for h4 in range(4):
                c8p = sm1([8, C])
                nc.tensor.transpose(c8p[:], cNt[:, h4 * 8:(h4 + 1) * 8], idf[:])
                cN8 = chk.tile([8, C], fp32, tag=f"cN8_{h4}", name=f"cN8_{h4}")
                nc.scalar.copy(cN8[:], c8p[:])
                cN8s.append(cN8)
            b2p = sm1([128, 4])
            nc.tensor.transpose(b2p[:], bsg[:, ts], idf[0:4, 0:4])
            beta2 = chk.tile([128, 4], fp32, tag="beta2", name="beta2")
            nc.scalar.copy(beta2[:], b2p[:])

            for h in range(4):
                cfp = sm1([128, C])
                nc.tensor.matmul(cfp[:], replt[:], cN8s[h][:],
                                 start=True, stop=True)
                clast = chk.tile([128, 1], fp32, tag="clast", name="clast")
                nc.scalar.copy(clast[:], cfp[:, C - 1:C])
                bful = chk.tile([128, C], bf16, tag="bful", name="bful")
                nc.scalar.activation(bful[:], cfp[:], Act.Exp)
                bC = chk.tile([128, 1], fp32, tag="bC", name="bC")
                nc.scalar.activation(bC[:], cfp[:, C - 1:C], Act.Exp)
                kendf = chk.tile([128, C], bf16, tag="kendf", name="kendf")
                nc.scalar.activation(kendf[:], cfp[:], Act.Exp, scale=-1.0,
                                     bias=clast[:, 0:1])
                Wt = chk.tile([128, C], bf16, tag="Wt", name="Wt")
                nc.vector.tensor_tensor(Wt[:], kb[h][:, ts], bful[:], op=Alu.mult)
                qtT = chk.tile([128, C], bf16, tag="qtT", name="qtT")
                nc.vector.tensor_tensor(qtT[:], qb[h][:, ts], bful[:], op=Alu.mult)
                kend = chk.tile([128, C], bf16, tag="kend", name="kend")
                nc.vector.tensor_tensor(kend[:], kb[h][:, ts], kendf[:], op=Alu.mult)

                bca = pbig.tile([128, 8 * C], fp32, tag="big", name="bca")
                for n in range(8):
                    nc.tensor.matmul(bca[:, n * C:(n + 1) * C],
                                     s8f[:, n * 128:(n + 1) * 128],
                                     cN8s[h][:], start=True, stop=True)
                eall = big.tile([128, 8 * C], bf16, tag="eall", name="eall")
                for n in range(8):
                    dtn = chk.tile([128, C], fp32, tag="dtn", name="dtn")
                    nc.vector.tensor_scalar(dtn[:],
                                            bca[:, n * C:(n + 1) * C],
                                            cNt[:, h * 8 + n:h * 8 + n + 1], 0.0,
                                            op0=Alu.subtract, op1=Alu.min)
                    nc.scalar.activation(eall[:, n * C:(n + 1) * C], dtn[:], Act.Exp)
                kmsk = []
                for n in range(8):
                    km = chk.tile([128, C], bf16, tag=f"km{n}", name=f"km{n}")
                    nc.scalar.mul(km[:], kb[h][:, ts], gmct[:, n:n + 1])
                    kmsk.append(km)

                def corr_mat(src, maskt, nm):
                    pall = pbig.tile([128, 8 * C], fp32, tag="big", name=f"p{nm}")
                    for n in range(8):
                        nc.tensor.matmul(pall[:, n * C:(n + 1) * C],
                                         kmsk[n][:], src[:, ts],
                                         start=True, stop=True)
                    prod = big.tile([128, 8 * C], bf16, tag="prod", name=f"pr{nm}")
                    nc.vector.tensor_tensor(prod[:], eall[:], pall[:], op=Alu.mult)
                    t4 = chk.tile([128, 4 * C], bf16, tag=f"{nm}4", name=f"{nm}4")
                    nc.vector.tensor_tensor(t4[:], prod[:, :4 * C], prod[:, 4 * C:],
                                            op=Alu.add)
                    t2 = chk.tile([128, 2 * C], bf16, tag=f"{nm}2", name=f"{nm}2")
                    nc.vector.tensor_tensor(t2[:], t4[:, :2 * C], t4[:, 2 * C:],
                                            op=Alu.add)
                    t1 = chk.tile([128, C], bf16, tag=f"{nm}1", name=f"{nm}1")
                    nc.vector.tensor_tensor(t1[:], t2[:, :C], t2[:, C:], op=Alu.add)
                    tm = chk.tile([128, C], bf16, tag=f"{nm}m", name=f"{nm}m")
                    nc.vector.tensor_tensor(tm[:], t1[:], maskt[:], op=Alu.mult)
                    return tm
                MtM = corr_mat(kb[h], mMt, "M")
                GtM = corr_mat(qb[h], mGt, "G")
                Nt0 = chk.tile([128, C], bf16, tag="Nt0", name="Nt0")
                nc.vector.tensor_scalar(Nt0[:], MtM[:], beta2[:, h:h + 1], None,
                                        op0=Alu.mult)

                vtp = sm1([128, C], bf16)
                nc.tensor.transpose(vtp[:], vb[h][:, ts], idb[:])
                vt = chk.tile([128, C], bf16, tag="vt", name="vt")
                nc.scalar.copy(vt[:], vtp[:])
                ws0 = sm1([128, C])
                nc.tensor.matmul(ws0[:], Wt[:], Sb[h][:], start=True, stop=True)
                xf = chk.tile([128, C], fp32, tag="xf", name="xf")
                nc.vector.tensor_tensor(xf[:], vt[:], ws0[:], op=Alu.subtract)
                xb = chk.tile([128, C], bf16, tag="xb", name="xb")
                nc.scalar.copy(xb[:], xf[:])

                Hs = [Nt0]
                g0p = psm.tile([128, C], bf16, tag="gp", bufs=1, name="g0p")
                nc.tensor.transpose(g0p[:], Nt0[:], idb[:])
                gsb = chk.tile([128, C], bf16, tag="gsb", name="gsb")
                nc.scalar.copy(gsb[:], g0p[:])
                for lev in range(6):
                    sqps = psm.tile([128, C], fp32, tag="gp", bufs=1, name="sqps")
                    nc.tensor.matmul(sqps[:], Hs[-1][:], gsb[:], start=True, stop=True)
                    gnew = chk.tile([128, C], bf16, tag="gsb", name="gnew")
                    nc.scalar.copy(gnew[:], sqps[:])
                    htp_ = psm.tile([128, C], bf16, tag="gp", bufs=1, name="htp_")
                    nc.tensor.transpose(htp_[:], gnew[:], idb[:])
                    hnew = chk.tile([128, C], bf16, tag=f"H{lev + 1}", name=f"H{lev + 1}")
                    nc.scalar.copy(hnew[:], htp_[:])
                    Hs.append(hnew)
                    gsb = gnew
                for lev in range(6, -1, -1):
                    mx = psm.tile([128, C], fp32, tag="gp", bufs=1, name="mx")
                    nc.tensor.matmul(mx[:], Hs[lev][:], xb[:], start=True, stop=True)
                    xf2 = chk.tile([128, C], fp32, tag="xf", name="xf2")
                    nc.vector.tensor_tensor(xf2[:], xf[:], mx[:],
                                            op=(Alu.add if lev > 0 else Alu.subtract))
                    xf = xf2
                    xb = chk.tile([128, C], bf16, tag="xb", name="xb2")
                    nc.scalar.copy(xb[:], xf[:])
                u = chk.tile([128, C], fp32, tag="u", name="u")
                nc.vector.tensor_scalar(u[:], xf[:], beta2[:, h:h + 1], None,
                                        op0=Alu.mult)
                ub = chk.tile([128, C], bf16, tag="ub", name="ub")
                nc.scalar.copy(ub[:], u[:])

                otp = psm.tile([128, C], fp32, tag="otp", bufs=1, name="otp")
                nc.tensor.matmul(otp[:], Sb[h][:], qtT[:], start=True, stop=False)
                nc.tensor.matmul(otp[:], ub[:], GtM[:], start=False, stop=True)

                ktp = sm1([128, C], bf16)
                nc.tensor.transpose(ktp[:], kend[:], idb[:])
                kts = chk.tile([128, C], bf16, tag="kts", name="kts")
                nc.scalar.copy(kts[:], ktp[:])
                sup = sm1([128, C])
                nc.tensor.matmul(sup[:], kts[:], ub[:], start=True, stop=True)
                nc.vector.scalar_tensor_tensor(Sf[h][:], Sf[h][:], bC[:, 0:1],
                                               sup[:], op0=Alu.mult, op1=Alu.add)
                nc.scalar.copy(Sb[h][:], Sf[h][:])

                yf = chk.tile([128, C], fp32, tag="yf", name="yf")
                nc.vector.tensor_tensor(yf[:], gateb[h][:, ts], otp[:], op=Alu.mult)
                ysq = chk.tile([128, C], bf16, tag="ysq", name="ysq")
                nc.scalar.activation(ysq[:], yf[:], Act.Square)
                ssp = sm1([1, C])
                nc.tensor.matmul(ssp[:], oct_[:], ysq[:], start=True, stop=True)
                nrc = chk.tile([1, C], fp32, tag="nrc", name="nrc")
                nc.scalar.activation(nrc[:], ssp[:], Act.Ln, scale=1.0 / DV,
                                     bias=epsnt[:, 0:1])
                rcb = chk.tile([1, C], bf16, tag="rcb", name="rcb")
                nc.scalar.activation(rcb[:], nrc[:], Act.Exp, scale=-0.5)
                rbc = sm1([128, C])
                nc.tensor.matmul(rbc[:], o1b[:], rcb[:], start=True, stop=True)
                nc.vector.scalar_tensor_tensor(yb[h][:, ts], yf[:], nwt[:, 0:1],
                                               rbc[:], op0=Alu.mult, op1=Alu.mult)

        # ---- output projection ----
        wot = [pers.tile([128, D], bf16, tag=f"wo{k}", name=f"wo{k}") for k in range(4)]
        for k in range(4):
            dma(wot[k][:], wo[k * 128:(k + 1) * 128, :])
        for m in range(16):
            for half in range(2):
                ps = pps.tile([128, 512], fp32, tag="proj", name="ops")
                for k in range(4):
                    nc.tensor.matmul(ps[:], wot[k][:, m * 128:(m + 1) * 128],
                                     yb[k][:, half * 512:(half + 1) * 512],
                                     start=(k == 0), stop=(k == 3))
                osb = convp.tile([128, 512], fp32, tag="osb", name="osb", bufs=2)
                nc.vector.tensor_copy(osb[:], ps[:])
                dma(outT[m * 128:(m + 1) * 128, half * 512:(half + 1) * 512], osb[:])

    nc.compile()
    return nc


def _prep_inputs(inputs):
    f32 = np.float32
    hs = np.asarray(inputs['hidden_states'], f32)
    maps = []
    tri = np.tril(np.ones((C, C), f32))
    maskM = (1.0 - tri).astype(BF)
    maskG = (1.0 - tri + np.eye(C, dtype=f32)).astype(BF)
    repl = np.zeros((NG, DK), f32)
    for n in range(NG):
        repl[n, n * GG:(n + 1) * GG] = 1.0
    sel8 = np.zeros((NG, NG * 128), f32)
    for n in range(NG):
        sel8[n, n * 128:(n + 1) * 128] = 1.0
    oh8 = np.zeros((DK, 64), f32)
    for i in range(8):
        oh8[:, i * 8 + i] = 1.0
    ident = np.eye(128, dtype=f32)
    for c in range(8):
        b, hg = c // 4, c % 4
        cols = slice(hg * NH * DK, (hg + 1) * NH * DK)
        gcols = slice(hg * NH * NG, (hg + 1) * NH * NG)
        hcols = slice(hg * NH, (hg + 1) * NH)
        nega = -np.exp(np.repeat(np.asarray(inputs['A_log'], f32)[hcols], NG))
        m = {
            'hT': np.ascontiguousarray(hs[b].T).astype(BF),
            'wq': np.asarray(inputs['Wq'], f32)[:, cols].astype(BF),
            'wk': np.asarray(inputs['Wk'], f32)[:, cols].astype(BF),
            'wv': np.asarray(inputs['Wv'], f32)[:, cols].astype(BF),
            'wg': np.asarray(inputs['Wg'], f32)[:, cols].astype(BF),
            'wo': np.asarray(inputs['Wo'], f32)[cols, :].astype(BF),
            'wf1': np.asarray(inputs['Wf1'], f32).astype(BF),
            'wf2': np.asarray(inputs['Wf2'], f32)[:, gcols].astype(BF),
            'wb': np.asarray(inputs['Wb'], f32)[:, hcols].astype(BF),
            'cw': np.ascontiguousarray(np.concatenate(
                [np.asarray(inputs['conv_q'], f32)[cols],
                 np.asarray(inputs['conv_k'], f32)[cols],
                 np.asarray(inputs['conv_v'], f32)[cols]], 1)),
            'nega': np.ascontiguousarray(nega[:, None]).astype(f32),
            'dtb': np.ascontiguousarray(
                np.asarray(inputs['dt_bias'], f32)[gcols][:, None]),
            'bgc': np.ascontiguousarray(
                np.asarray(inputs['bg'], f32)[cols].reshape(NH, DV).T),
            'normw': np.ascontiguousarray(
                np.asarray(inputs['norm_w'], f32)[:, None]),
            'repl': repl,
            'self8f': sel8,
            'sel8b': sel8.astype(BF),
            'gmc': np.ascontiguousarray(repl.T),
            'ones1b': np.ones((1, C), f32).astype(BF),
            'onescol': np.ones((DK, 1), f32).astype(BF),
            'oh8': oh8.astype(BF),
            'sc8': np.array([[1.0 / SCALE ** 2]] * 4 + [[1.0]] * 4, f32),
            'eps8': np.array([[1e-6 / SCALE ** 2]] * 4 + [[1e-6]] * 4, f32),
            'epsn': np.array([[EPS]], f32),
            'maskM': maskM,
            'maskG': maskG,
            'idbf': ident.astype(BF),
            'idf32': ident,
        }
        maps.append(m)
    return maps


def kernel(**inputs):
    from concourse.bass_utils import run_bass_kernel_spmd
    if 'nc' not in _CACHE:
        _CACHE['nc'] = _build()
    nc = _CACHE['nc']
    maps = _prep_inputs(inputs)
    res = run_bass_kernel_spmd(nc, maps, list(range(8))).results
    out = np.zeros((B, T, D), np.float32)
    for c in range(8):
        out[c // 4] += res[c]['outT'].T.astype(np.float32)
    return out

